# revision 25
# baseline (speedup 1.0000x reference)
"""MultiHeadDualAttention Trainium2 kernel.

Sharding: 8 heads -> 8 cores (tensor parallel over heads). Each core gets the
full k1/v1/k2/v2 (pre-transposed on host to [256, 4096] so the contraction dim
lands on SBUF partitions) plus its head's slices of the wk/wv/wo weights.

Math per head (verified exact vs reference in fp64):
  o2 = rowsoftmax(S_true) @ v2p_full ; o1 = colsoftmax(S_true)^T @ v1p_full
  - v-bias cancels through softmax row-sums == 1, re-added on host via
    (bv @ wo + bo) constants.
  - k-bias: rowsoftmax(S_true) == rowsoftmax(k1p_FULL @ k2p_NOB^T) and
    colsoftmax(S_true) == colsoftmax(k1p_NOB @ k2p_FULL^T), so each direction
    uses one biased and one unbiased projection and no rank-1 corrections.
  - exp without max-subtraction: |SCALE*S| < ~2.5, safe in fp32.
  - softmax denominators exported unnormalized (den1/den2); host divides.

Perf structure:
  - wk weights are shipped column-duplicated [256, 128] so the k projections
    land duplicated on both partition halves; the K=64 score matmuls then run
    2x row-packed (64x128 PE tiles T0/T8 via base_partition 0/64).
  - E is built in [128, 16, 1024] bf16 sub-blocks (exp at FD=1024 amortizes
    the ACT instruction overhead), double-buffered against the PV matmuls.
  - PV uses a ones-augmented V (M=65) accumulating [65, 512] in PSUM over all
    32 partition-tiles; row 64 is the softmax denominator.
Output projection wo is applied on-device per head ([64,256] slice); host sums
the 8 partial [256, 4096] results (the "all-reduce" of the row-sharded wo).
"""

import sys

sys.path.insert(0, "/opt/trn_rl_repo")

import numpy as np

N = 4096
C = 256
AD = 512
H = 8
D = 64
SCALE = float(D) ** -0.5
NCORES = 8
NBLK = 1024         # free-dim block width for E
NCHUNK = N // NBLK  # 4 blocks
MT = N // 128       # 32 partition-tiles of E per block
SUB = 4             # m-tiles per E sub-block

_cache: dict = {}


def _build_module():
    import concourse.bacc as bacc
    import concourse.mybir as mybir
    import concourse.tile as tile

    f32 = mybir.dt.float32
    bf16 = mybir.dt.bfloat16
    Exp = mybir.ActivationFunctionType.Exp

    nc = bacc.Bacc("TRN2", target_bir_lowering=False, debug=False)

    def din(name, shape, dt=bf16):
        return nc.dram_tensor(name, shape, dt, kind="ExternalInput").ap()

    def dout(name, shape):
        return nc.dram_tensor(name, shape, f32, kind="ExternalOutput").ap()

    k1T = din("k1T", [C, N])
    v1T = din("v1T", [C, N])
    k2T = din("k2T", [C, N])
    v2T = din("v2T", [C, N])
    wk1 = din("wk1", [C, 128])   # column-duplicated [wk|wk]
    wk2 = din("wk2", [C, 128])
    wv1 = din("wv1", [C, D])
    wv2 = din("wv2", [C, D])
    bk1 = din("bk1", [128, 1], f32)  # row-duplicated
    bk2 = din("bk2", [128, 1], f32)
    wo1 = din("wo1", [128, C])  # row-duplicated [wo; wo]
    wo2 = din("wo2", [128, C])

    o1pT = dout("o1pT", [C, N])
    o2pT = dout("o2pT", [C, N])
    den1 = dout("den1", [128, MT])   # den1[m] at [m%128, m//128]
    den2 = dout("den2", [128, MT])

    with tile.TileContext(nc) as tc:
        with (
            tc.tile_pool(name="const", bufs=1) as constp,
            tc.tile_pool(name="raw", bufs=12) as rawp,
            tc.tile_pool(name="eblk", bufs=8) as ep,
            tc.tile_pool(name="outp", bufs=3) as outp,
            tc.tile_pool(name="spsum", bufs=2, space="PSUM") as spsum,
            tc.tile_pool(name="opsum", bufs=4, space="PSUM") as opsum,
        ):
            # ---- PE warm-up: ~7us of dummy matmuls so the HAM clock-gate
            # reaches K=8/8 (2.4 GHz) before the real work arrives ----
            warm = constp.tile([128, 512], bf16, tag="warm")
            nc.gpsimd.memset(warm[:], 0.0)
            wps = opsum.tile([128, 512], f32, tag="op", name="warm_ps")
            for _ in range(16):
                nc.tensor.matmul(wps[:], warm[:, 0:128], warm[:], start=True, stop=True)

            # ---- load weights ----
            w_sb = {}
            for name, drt, w in (("wk1", wk1, 128), ("wk2", wk2, 128),
                                 ("wv1", wv1, D), ("wv2", wv2, D)):
                t = constp.tile([128, 2, w], bf16, tag=name)
                for ct in range(2):
                    nc.sync.dma_start(out=t[:, ct, :], in_=drt[ct * 128:(ct + 1) * 128, :])
                w_sb[name] = t
            bk1_sb = constp.tile([128, 1], f32, tag="bk1")
            nc.sync.dma_start(out=bk1_sb[:], in_=bk1[:])
            bk2_sb = constp.tile([128, 1], f32, tag="bk2")
            nc.sync.dma_start(out=bk2_sb[:], in_=bk2[:])
            wo1_sb = constp.tile([128, C], bf16, tag="wo1")
            nc.sync.dma_start(out=wo1_sb[:], in_=wo1[:])
            wo2_sb = constp.tile([128, C], bf16, tag="wo2")
            nc.sync.dma_start(out=wo2_sb[:], in_=wo2[:])

            # ---- k projections: [128, 4096] bf16, data duplicated on both
            # partition halves (weights are column-duplicated) ----
            def k_proj(rawT, w, b_sb, tagbase):
                # chunked tiles so score matmuls can start before the whole
                # projection finishes (Tile deps are per-tile)
                full = []
                for j in range(8):
                    raw = rawp.tile([128, 2, 512], bf16, tag="raw")
                    for ct in range(2):
                        nc.sync.dma_start(
                            out=raw[:, ct, :],
                            in_=rawT[ct * 128:(ct + 1) * 128, j * 512:(j + 1) * 512],
                        )
                    ps = opsum.tile([128, 512], f32, tag="op")
                    for ct in range(2):
                        nc.tensor.matmul(
                            ps[:], w[:, ct, :], raw[:, ct, :],
                            start=(ct == 0), stop=(ct == 1),
                        )
                    fullj = constp.tile([128, 512], bf16, tag=f"{tagbase}_full{j}")
                    nc.vector.tensor_scalar_add(fullj[:], ps[:], b_sb[:])
                    full.append(fullj)
                return full

            k1_full = k_proj(k1T, w_sb["wk1"], bk1_sb, "k1p")
            k2_full = k_proj(k2T, w_sb["wk2"], bk2_sb, "k2p")

            # ---- v projections: [128, 32, 64] bf16 ----
            def v_proj(rawT, w, tagbase):
                vaug = constp.tile([128, MT, D], bf16, tag=tagbase)
                for j in range(8):
                    raw = rawp.tile([128, 2, 512], bf16, tag="raw")
                    for ct in range(2):
                        nc.sync.dma_start(
                            out=raw[:, ct, :],
                            in_=rawT[ct * 128:(ct + 1) * 128, j * 512:(j + 1) * 512],
                        )
                    for k in range(4):
                        nt = j * 4 + k
                        ps = opsum.tile([128, D], f32, tag="op")
                        for ct in range(2):
                            nc.tensor.matmul(
                                ps[:], raw[:, ct, k * 128:(k + 1) * 128],
                                w[:, ct, :],
                                start=(ct == 0), stop=(ct == 1),
                            )
                        nc.vector.tensor_copy(vaug[:, nt, :D], ps[:])
                return vaug


            # ---- one softmax direction ----
            def attention_pass(kP, kF, vaug, oT_tag):
                """E[p, f] = exp(SCALE * kP[:,p]^T kF[:,f]); oT = vaug^T E.

                Score matmuls run 2x row-packed (64x128 tiles T0/T8). The exp
                accum_out sums E along the free axis, which is the OTHER
                direction's softmax denominator (both passes share the fully
                biased S). PV matmuls run 2x column-packed: even partition
                tiles accumulate into PSUM rows 0:64, odd into 64:128; the
                halves are summed later inside the output projection.
                """
                oT = []
                acc = constp.tile([128, MT, NCHUNK], f32, tag=oT_tag + "_acc")
                pvq = []

                def emit_pv(job):
                    jj, po, eblk, sub = job
                    for c in range(2):
                        for mtl in range(SUB):
                            mt = sub * SUB + mtl
                            half = (mt % 2) * 64
                            nc.tensor.matmul(
                                po[c][mt % 2][half:half + 64, :], vaug[:, mt, :],
                                eblk[:, mtl, c * 512:(c + 1) * 512],
                                start=(mt < 2), stop=(mt >= MT - 2),
                            )
                    if sub == 1:
                        for c in range(2):
                            oc = constp.tile([128, 512], bf16,
                                             tag=f"{oT_tag}_o{2*jj+c}")
                            nc.vector.tensor_copy(oc[0:64, :], po[c][0][0:64, :])
                            nc.vector.tensor_copy(oc[64:128, :], po[c][1][64:128, :])
                            oT.append(oc)

                for j in range(NCHUNK):
                    po = [[opsum.tile([128, 512], f32, tag="op",
                                      name=f"po_{oT_tag}_{j}_{c}_{h}")
                           for h in range(2)] for c in range(2)]
                    for sub in range(MT // SUB):
                        eblk = ep.tile([128, SUB, NBLK], bf16, tag="eblk")
                        for pair in range(SUB // 2):
                            mtA = sub * SUB + 2 * pair
                            mtB = mtA + 1
                            psA = spsum.tile([128, NBLK], f32, tag="sp")
                            psB = spsum.tile([128, NBLK], f32, tag="sp")
                            for ps, mt, lo, hi in ((psA, mtA, 0, 64), (psB, mtB, 64, 128)):
                                for c in range(2):
                                    nc.tensor.matmul(
                                        ps[:, c * 512:(c + 1) * 512],
                                        kP[mt // 4][lo:hi, (mt % 4) * 128:(mt % 4 + 1) * 128],
                                        kF[2 * j + c][lo:hi, :],
                                        start=True, stop=True,
                                    )
                            nc.scalar.activation(eblk[:, 2 * pair, :], psA[:], Exp,
                                                 scale=SCALE,
                                                 accum_out=acc[:, mtA, j:j + 1])
                            nc.scalar.activation(eblk[:, 2 * pair + 1, :], psB[:], Exp,
                                                 scale=SCALE,
                                                 accum_out=acc[:, mtB, j:j + 1])
                        pvq.append((j, po, eblk, sub))
                        if len(pvq) > 1:
                            emit_pv(pvq.pop(0))
                while pvq:
                    emit_pv(pvq.pop(0))
                # reduce the per-block accum partials -> other direction's den
                accr = constp.tile([128, MT], f32, tag=oT_tag + "_accr")
                nc.vector.tensor_reduce(accr[:], acc[:], mybir.AxisListType.X,
                                        mybir.AluOpType.add)
                return oT, accr

            # ---- output projections (per-head slice of wo); the two PV
            # halves (PSUM rows 0:64 / 64:128) are summed here via two
            # accumulating row-group matmuls against row-duplicated wo ----
            def out_proj(oT, accr, wo_sb, outdr, dendr):
                for j in range(8):
                    for ct in range(2):
                        pp = opsum.tile([128, 512], f32, tag="op")
                        nc.tensor.matmul(
                            pp[:], wo_sb[0:64, ct * 128:(ct + 1) * 128],
                            oT[j][0:64, :], start=True, stop=False,
                        )
                        nc.tensor.matmul(
                            pp[:], wo_sb[64:128, ct * 128:(ct + 1) * 128],
                            oT[j][64:128, :], start=False, stop=True,
                        )
                        ot = outp.tile([128, 512], f32, tag="out")
                        nc.vector.tensor_copy(ot[:], pp[:])
                        nc.sync.dma_start(
                            out=outdr[ct * 128:(ct + 1) * 128, j * 512:(j + 1) * 512],
                            in_=ot[:],
                        )
                nc.sync.dma_start(out=dendr[:], in_=accr[:])

            v1_aug = v_proj(v1T, w_sb["wv1"], "v1aug")
            v2_aug = v_proj(v2T, w_sb["wv2"], "v2aug")

            # o2 pass: E[m, n]; its exp accums give den1 (sum over n)
            o2T, acc1 = attention_pass(k2_full, k1_full, v2_aug, "o2T")
            out_proj(o2T, acc1, wo2_sb, o2pT, den1)
            # o1 pass: E[n, m]; its exp accums give den2 (sum over m)
            o1T, acc2 = attention_pass(k1_full, k2_full, v1_aug, "o1T")
            out_proj(o1T, acc2, wo1_sb, o1pT, den2)

    nc.compile()
    return nc


def _get_nc():
    if "nc" not in _cache:
        _cache["nc"] = _build_module()
    return _cache["nc"]


def kernel(k1, v1, k2, v2,
           wk1_w, wk1_b, wv1_w, wv1_b,
           wk2_w, wk2_b, wv2_w, wv2_b,
           wo1_w, wo1_b, wo2_w, wo2_b):
    import ml_dtypes
    from concourse.bass_utils import run_bass_kernel_spmd

    nc = _get_nc()

    f = np.float32
    bf = ml_dtypes.bfloat16
    k1T = np.ascontiguousarray(np.asarray(k1, f).T).astype(bf)
    v1T = np.ascontiguousarray(np.asarray(v1, f).T).astype(bf)
    k2T = np.ascontiguousarray(np.asarray(k2, f).T).astype(bf)
    v2T = np.ascontiguousarray(np.asarray(v2, f).T).astype(bf)

    def dup2(a):  # [C, D] -> [C, 128] column-duplicated
        return np.ascontiguousarray(np.concatenate([a, a], axis=1))

    in_maps = []
    for h in range(NCORES):
        sl = slice(h * D, (h + 1) * D)
        in_maps.append({
            "k1T": k1T, "v1T": v1T, "k2T": k2T, "v2T": v2T,
            "wk1": dup2(np.asarray(wk1_w, f)[:, sl]).astype(bf),
            "wv1": np.ascontiguousarray(np.asarray(wv1_w, f)[:, sl]).astype(bf),
            "wk2": dup2(np.asarray(wk2_w, f)[:, sl]).astype(bf),
            "wv2": np.ascontiguousarray(np.asarray(wv2_w, f)[:, sl]).astype(bf),
            "bk1": np.ascontiguousarray(np.tile(np.asarray(wk1_b, f)[sl].reshape(D, 1), (2, 1))),
            "bk2": np.ascontiguousarray(np.tile(np.asarray(wk2_b, f)[sl].reshape(D, 1), (2, 1))),
            "wo1": np.ascontiguousarray(np.concatenate([np.asarray(wo1_w, f)[sl, :]] * 2, axis=0)).astype(bf),
            "wo2": np.ascontiguousarray(np.concatenate([np.asarray(wo2_w, f)[sl, :]] * 2, axis=0)).astype(bf),
        })

    res = run_bass_kernel_spmd(nc, in_maps, list(range(NCORES)))
    _cache["last_result"] = res

    out1 = np.zeros((N, C), np.float32)
    out2 = np.zeros((N, C), np.float32)
    for h in range(NCORES):
        rh = res.results[h]
        den1v = rh["den1"].T.reshape(N)   # den1[m], m = mt*128 + p
        den2v = rh["den2"].T.reshape(N)   # den2[n]
        out1 += (rh["o1pT"] / den1v[None, :]).T
        out2 += (rh["o2pT"] / den2v[None, :]).T
    # v-bias and output bias constants (v-bias commutes through softmax).
    out1 += np.asarray(wv1_b, f) @ np.asarray(wo1_w, f) + np.asarray(wo1_b, f)
    out2 += np.asarray(wv2_b, f) @ np.asarray(wo2_w, f) + np.asarray(wo2_b, f)
    return out1, out2


# revision 26
# speedup vs baseline: 1.0214x; 1.0214x over previous
"""MultiHeadDualAttention Trainium2 kernel.

Sharding: 8 heads -> 8 cores (tensor parallel over heads). Each core gets the
full k1/v1/k2/v2 (pre-transposed on host to [256, 4096] so the contraction dim
lands on SBUF partitions) plus its head's slices of the wk/wv/wo weights.

Math per head (verified exact vs reference in fp64):
  o2 = rowsoftmax(S_true) @ v2p_full ; o1 = colsoftmax(S_true)^T @ v1p_full
  - v-bias cancels through softmax row-sums == 1, re-added on host via
    (bv @ wo + bo) constants.
  - k-bias: rowsoftmax(S_true) == rowsoftmax(k1p_FULL @ k2p_NOB^T) and
    colsoftmax(S_true) == colsoftmax(k1p_NOB @ k2p_FULL^T), so each direction
    uses one biased and one unbiased projection and no rank-1 corrections.
  - exp without max-subtraction: |SCALE*S| < ~2.5, safe in fp32.
  - softmax denominators exported unnormalized (den1/den2); host divides.

Perf structure:
  - wk weights are shipped column-duplicated [256, 128] so the k projections
    land duplicated on both partition halves; the K=64 score matmuls then run
    2x row-packed (64x128 PE tiles T0/T8 via base_partition 0/64).
  - E is built in [128, 16, 1024] bf16 sub-blocks (exp at FD=1024 amortizes
    the ACT instruction overhead), double-buffered against the PV matmuls.
  - PV uses a ones-augmented V (M=65) accumulating [65, 512] in PSUM over all
    32 partition-tiles; row 64 is the softmax denominator.
Output projection wo is applied on-device per head ([64,256] slice); host sums
the 8 partial [256, 4096] results (the "all-reduce" of the row-sharded wo).
"""

import sys

sys.path.insert(0, "/opt/trn_rl_repo")

import numpy as np

N = 4096
C = 256
AD = 512
H = 8
D = 64
SCALE = float(D) ** -0.5
NCORES = 8
NBLK = 1024         # free-dim block width for E
NCHUNK = N // NBLK  # 4 blocks
MT = N // 128       # 32 partition-tiles of E per block
SUB = 4             # m-tiles per E sub-block

_cache: dict = {}


def _build_module():
    import concourse.bacc as bacc
    import concourse.mybir as mybir
    import concourse.tile as tile

    f32 = mybir.dt.float32
    bf16 = mybir.dt.bfloat16
    Exp = mybir.ActivationFunctionType.Exp

    nc = bacc.Bacc("TRN2", target_bir_lowering=False, debug=False)

    def din(name, shape, dt=bf16):
        return nc.dram_tensor(name, shape, dt, kind="ExternalInput").ap()

    def dout(name, shape):
        return nc.dram_tensor(name, shape, f32, kind="ExternalOutput").ap()

    k1T = din("k1T", [C, N])
    v1T = din("v1T", [C, N])
    k2T = din("k2T", [C, N])
    v2T = din("v2T", [C, N])
    wk1 = din("wk1", [C, 128])   # column-duplicated [wk|wk]
    wk2 = din("wk2", [C, 128])
    wv1 = din("wv1", [C, D])
    wv2 = din("wv2", [C, D])
    bk1 = din("bk1", [128, 1], f32)  # row-duplicated
    bk2 = din("bk2", [128, 1], f32)
    wo1 = din("wo1", [128, C])  # row-duplicated [wo; wo]
    wo2 = din("wo2", [128, C])

    o1pT = dout("o1pT", [C, N])
    o2pT = dout("o2pT", [C, N])
    den1 = dout("den1", [128, MT])   # den1[m] at [m%128, m//128]
    den2 = dout("den2", [128, MT])

    with tile.TileContext(nc) as tc:
        with (
            tc.tile_pool(name="const", bufs=1) as constp,
            tc.tile_pool(name="raw", bufs=8) as rawp,
            tc.tile_pool(name="eblk", bufs=8) as ep,
            tc.tile_pool(name="outp", bufs=3) as outp,
            tc.tile_pool(name="spsum", bufs=2, space="PSUM") as spsum,
            tc.tile_pool(name="opsum", bufs=4, space="PSUM") as opsum,
        ):
            # ---- PE warm-up: ~7us of dummy matmuls so the HAM clock-gate
            # reaches K=8/8 (2.4 GHz) before the real work arrives ----
            warm = constp.tile([128, 512], bf16, tag="warm")
            nc.gpsimd.memset(warm[:], 0.0)
            wps = opsum.tile([128, 512], f32, tag="op", name="warm_ps")
            for _ in range(16):
                nc.tensor.matmul(wps[:], warm[:, 0:128], warm[:], start=True, stop=True)

            # ---- load weights ----
            w_sb = {}
            for name, drt, w in (("wk1", wk1, 128), ("wk2", wk2, 128),
                                 ("wv1", wv1, D), ("wv2", wv2, D)):
                t = constp.tile([128, 2, w], bf16, tag=name)
                for ct in range(2):
                    nc.sync.dma_start(out=t[:, ct, :], in_=drt[ct * 128:(ct + 1) * 128, :])
                w_sb[name] = t
            bk1_sb = constp.tile([128, 1], f32, tag="bk1")
            nc.sync.dma_start(out=bk1_sb[:], in_=bk1[:])
            bk2_sb = constp.tile([128, 1], f32, tag="bk2")
            nc.sync.dma_start(out=bk2_sb[:], in_=bk2[:])
            wo1_sb = constp.tile([128, C], bf16, tag="wo1")
            nc.sync.dma_start(out=wo1_sb[:], in_=wo1[:])
            wo2_sb = constp.tile([128, C], bf16, tag="wo2")
            nc.sync.dma_start(out=wo2_sb[:], in_=wo2[:])

            # ---- k projections: [128, 4096] bf16, data duplicated on both
            # partition halves (weights are column-duplicated) ----
            def k_proj(rawT, w, b_sb, tagbase):
                # chunked tiles so score matmuls can start before the whole
                # projection finishes (Tile deps are per-tile)
                full = []
                for j in range(8):
                    raw = rawp.tile([128, 2, 512], bf16, tag="raw")
                    for ct in range(2):
                        nc.sync.dma_start(
                            out=raw[:, ct, :],
                            in_=rawT[ct * 128:(ct + 1) * 128, j * 512:(j + 1) * 512],
                        )
                    ps = opsum.tile([128, 512], f32, tag="op")
                    for ct in range(2):
                        nc.tensor.matmul(
                            ps[:], w[:, ct, :], raw[:, ct, :],
                            start=(ct == 0), stop=(ct == 1),
                        )
                    fullj = constp.tile([128, 512], bf16, tag=f"{tagbase}_full{j}")
                    nc.vector.tensor_scalar_add(fullj[:], ps[:], b_sb[:])
                    full.append(fullj)
                return full

            k1_full = k_proj(k1T, w_sb["wk1"], bk1_sb, "k1p")
            k2_full = k_proj(k2T, w_sb["wk2"], bk2_sb, "k2p")

            # ---- v projections: [128, 32, 64] bf16 ----
            def v_proj(rawT, w, tagbase):
                vaug = constp.tile([128, MT, D], bf16, tag=tagbase)
                for j in range(8):
                    raw = rawp.tile([128, 2, 512], bf16, tag="raw")
                    for ct in range(2):
                        nc.sync.dma_start(
                            out=raw[:, ct, :],
                            in_=rawT[ct * 128:(ct + 1) * 128, j * 512:(j + 1) * 512],
                        )
                    for k in range(4):
                        nt = j * 4 + k
                        ps = opsum.tile([128, D], f32, tag="op")
                        for ct in range(2):
                            nc.tensor.matmul(
                                ps[:], raw[:, ct, k * 128:(k + 1) * 128],
                                w[:, ct, :],
                                start=(ct == 0), stop=(ct == 1),
                            )
                        nc.vector.tensor_copy(vaug[:, nt, :D], ps[:])
                return vaug


            # ---- one softmax direction ----
            def attention_pass(kP, kF, vaug, oT_tag):
                """E[p, f] = exp(SCALE * kP[:,p]^T kF[:,f]); oT = vaug^T E.

                Score matmuls run 2x row-packed (64x128 tiles T0/T8). The exp
                accum_out sums E along the free axis, which is the OTHER
                direction's softmax denominator (both passes share the fully
                biased S). PV matmuls run 2x column-packed: even partition
                tiles accumulate into PSUM rows 0:64, odd into 64:128; the
                halves are summed later inside the output projection.
                """
                oT = []
                acc = constp.tile([128, MT, NCHUNK], f32, tag=oT_tag + "_acc")
                pvq = []

                def emit_pv(job):
                    jj, po, eblk, sub = job
                    for c in range(2):
                        for mtl in range(SUB):
                            mt = sub * SUB + mtl
                            half = (mt % 2) * 64
                            nc.tensor.matmul(
                                po[c][mt % 2][half:half + 64, :], vaug[:, mt, :],
                                eblk[:, mtl, c * 512:(c + 1) * 512],
                                start=(mt < 2), stop=(mt >= MT - 2),
                            )
                    if sub == 1:
                        for c in range(2):
                            oc = constp.tile([128, 512], bf16,
                                             tag=f"{oT_tag}_o{2*jj+c}")
                            nc.vector.tensor_copy(oc[0:64, :], po[c][0][0:64, :])
                            nc.vector.tensor_copy(oc[64:128, :], po[c][1][64:128, :])
                            oT.append(oc)

                for j in range(NCHUNK):
                    po = [[opsum.tile([128, 512], f32, tag="op",
                                      name=f"po_{oT_tag}_{j}_{c}_{h}")
                           for h in range(2)] for c in range(2)]
                    for sub in range(MT // SUB):
                        eblk = ep.tile([128, SUB, NBLK], bf16, tag="eblk")
                        for pair in range(SUB // 2):
                            mtA = sub * SUB + 2 * pair
                            mtB = mtA + 1
                            psA = spsum.tile([128, NBLK], f32, tag="sp")
                            psB = spsum.tile([128, NBLK], f32, tag="sp")
                            for ps, mt, lo, hi in ((psA, mtA, 0, 64), (psB, mtB, 64, 128)):
                                for c in range(2):
                                    nc.tensor.matmul(
                                        ps[:, c * 512:(c + 1) * 512],
                                        kP[mt // 4][lo:hi, (mt % 4) * 128:(mt % 4 + 1) * 128],
                                        kF[2 * j + c][lo:hi, :],
                                        start=True, stop=True,
                                    )
                            nc.scalar.activation(eblk[:, 2 * pair, :], psA[:], Exp,
                                                 scale=SCALE,
                                                 accum_out=acc[:, mtA, j:j + 1])
                            nc.scalar.activation(eblk[:, 2 * pair + 1, :], psB[:], Exp,
                                                 scale=SCALE,
                                                 accum_out=acc[:, mtB, j:j + 1])
                        pvq.append((j, po, eblk, sub))
                        if len(pvq) > 1:
                            emit_pv(pvq.pop(0))
                while pvq:
                    emit_pv(pvq.pop(0))
                # reduce the per-block accum partials -> other direction's den
                accr = constp.tile([128, MT], f32, tag=oT_tag + "_accr")
                nc.vector.tensor_reduce(accr[:], acc[:], mybir.AxisListType.X,
                                        mybir.AluOpType.add)
                return oT, accr

            # ---- output projections (per-head slice of wo); the two PV
            # halves (PSUM rows 0:64 / 64:128) are summed here via two
            # accumulating row-group matmuls against row-duplicated wo ----
            def out_proj(oT, accr, wo_sb, outdr, dendr):
                for j in range(8):
                    for ct in range(2):
                        pp = opsum.tile([128, 512], f32, tag="op")
                        nc.tensor.matmul(
                            pp[:], wo_sb[0:64, ct * 128:(ct + 1) * 128],
                            oT[j][0:64, :], start=True, stop=False,
                        )
                        nc.tensor.matmul(
                            pp[:], wo_sb[64:128, ct * 128:(ct + 1) * 128],
                            oT[j][64:128, :], start=False, stop=True,
                        )
                        ot = outp.tile([128, 512], f32, tag="out")
                        nc.vector.tensor_copy(ot[:], pp[:])
                        nc.sync.dma_start(
                            out=outdr[ct * 128:(ct + 1) * 128, j * 512:(j + 1) * 512],
                            in_=ot[:],
                        )
                nc.sync.dma_start(out=dendr[:], in_=accr[:])

            v1_aug = v_proj(v1T, w_sb["wv1"], "v1aug")
            v2_aug = v_proj(v2T, w_sb["wv2"], "v2aug")

            # o2 pass: E[m, n]; its exp accums give den1 (sum over n)
            o2T, acc1 = attention_pass(k2_full, k1_full, v2_aug, "o2T")
            out_proj(o2T, acc1, wo2_sb, o2pT, den1)
            # o1 pass: E[n, m]; its exp accums give den2 (sum over m)
            o1T, acc2 = attention_pass(k1_full, k2_full, v1_aug, "o1T")
            out_proj(o1T, acc2, wo1_sb, o1pT, den2)

    nc.compile()
    return nc


def _get_nc():
    if "nc" not in _cache:
        _cache["nc"] = _build_module()
    return _cache["nc"]


def kernel(k1, v1, k2, v2,
           wk1_w, wk1_b, wv1_w, wv1_b,
           wk2_w, wk2_b, wv2_w, wv2_b,
           wo1_w, wo1_b, wo2_w, wo2_b):
    import ml_dtypes
    from concourse.bass_utils import run_bass_kernel_spmd

    nc = _get_nc()

    f = np.float32
    bf = ml_dtypes.bfloat16
    k1T = np.ascontiguousarray(np.asarray(k1, f).T).astype(bf)
    v1T = np.ascontiguousarray(np.asarray(v1, f).T).astype(bf)
    k2T = np.ascontiguousarray(np.asarray(k2, f).T).astype(bf)
    v2T = np.ascontiguousarray(np.asarray(v2, f).T).astype(bf)

    def dup2(a):  # [C, D] -> [C, 128] column-duplicated
        return np.ascontiguousarray(np.concatenate([a, a], axis=1))

    in_maps = []
    for h in range(NCORES):
        sl = slice(h * D, (h + 1) * D)
        in_maps.append({
            "k1T": k1T, "v1T": v1T, "k2T": k2T, "v2T": v2T,
            "wk1": dup2(np.asarray(wk1_w, f)[:, sl]).astype(bf),
            "wv1": np.ascontiguousarray(np.asarray(wv1_w, f)[:, sl]).astype(bf),
            "wk2": dup2(np.asarray(wk2_w, f)[:, sl]).astype(bf),
            "wv2": np.ascontiguousarray(np.asarray(wv2_w, f)[:, sl]).astype(bf),
            "bk1": np.ascontiguousarray(np.tile(np.asarray(wk1_b, f)[sl].reshape(D, 1), (2, 1))),
            "bk2": np.ascontiguousarray(np.tile(np.asarray(wk2_b, f)[sl].reshape(D, 1), (2, 1))),
            "wo1": np.ascontiguousarray(np.concatenate([np.asarray(wo1_w, f)[sl, :]] * 2, axis=0)).astype(bf),
            "wo2": np.ascontiguousarray(np.concatenate([np.asarray(wo2_w, f)[sl, :]] * 2, axis=0)).astype(bf),
        })

    res = run_bass_kernel_spmd(nc, in_maps, list(range(NCORES)))
    _cache["last_result"] = res

    out1 = np.zeros((N, C), np.float32)
    out2 = np.zeros((N, C), np.float32)
    for h in range(NCORES):
        rh = res.results[h]
        den1v = rh["den1"].T.reshape(N)   # den1[m], m = mt*128 + p
        den2v = rh["den2"].T.reshape(N)   # den2[n]
        out1 += (rh["o1pT"] / den1v[None, :]).T
        out2 += (rh["o2pT"] / den2v[None, :]).T
    # v-bias and output bias constants (v-bias commutes through softmax).
    out1 += np.asarray(wv1_b, f) @ np.asarray(wo1_w, f) + np.asarray(wo1_b, f)
    out2 += np.asarray(wv2_b, f) @ np.asarray(wo2_w, f) + np.asarray(wo2_b, f)
    return out1, out2


# revision 27
# speedup vs baseline: 1.0265x; 1.0051x over previous
"""MultiHeadDualAttention Trainium2 kernel.

Sharding: 8 heads -> 8 cores (tensor parallel over heads). Each core gets the
full k1/v1/k2/v2 (pre-transposed on host to [256, 4096] so the contraction dim
lands on SBUF partitions) plus its head's slices of the wk/wv/wo weights.

Math per head (verified exact vs reference in fp64):
  o2 = rowsoftmax(S_true) @ v2p_full ; o1 = colsoftmax(S_true)^T @ v1p_full
  - v-bias cancels through softmax row-sums == 1, re-added on host via
    (bv @ wo + bo) constants.
  - k-bias: rowsoftmax(S_true) == rowsoftmax(k1p_FULL @ k2p_NOB^T) and
    colsoftmax(S_true) == colsoftmax(k1p_NOB @ k2p_FULL^T), so each direction
    uses one biased and one unbiased projection and no rank-1 corrections.
  - exp without max-subtraction: |SCALE*S| < ~2.5, safe in fp32.
  - softmax denominators exported unnormalized (den1/den2); host divides.

Perf structure:
  - wk weights are shipped column-duplicated [256, 128] so the k projections
    land duplicated on both partition halves; the K=64 score matmuls then run
    2x row-packed (64x128 PE tiles T0/T8 via base_partition 0/64).
  - E is built in [128, 16, 1024] bf16 sub-blocks (exp at FD=1024 amortizes
    the ACT instruction overhead), double-buffered against the PV matmuls.
  - PV uses a ones-augmented V (M=65) accumulating [65, 512] in PSUM over all
    32 partition-tiles; row 64 is the softmax denominator.
Output projection wo is applied on-device per head ([64,256] slice); host sums
the 8 partial [256, 4096] results (the "all-reduce" of the row-sharded wo).
"""

import sys

sys.path.insert(0, "/opt/trn_rl_repo")

import numpy as np

N = 4096
C = 256
AD = 512
H = 8
D = 64
SCALE = float(D) ** -0.5
NCORES = 8
NBLK = 1024         # free-dim block width for E
NCHUNK = N // NBLK  # 4 blocks
MT = N // 128       # 32 partition-tiles of E per block
SUB = 4             # m-tiles per E sub-block

_cache: dict = {}


def _build_module():
    import concourse.bacc as bacc
    import concourse.mybir as mybir
    import concourse.tile as tile

    f32 = mybir.dt.float32
    bf16 = mybir.dt.bfloat16
    Exp = mybir.ActivationFunctionType.Exp

    nc = bacc.Bacc("TRN2", target_bir_lowering=False, debug=False)

    def din(name, shape, dt=bf16):
        return nc.dram_tensor(name, shape, dt, kind="ExternalInput").ap()

    def dout(name, shape):
        return nc.dram_tensor(name, shape, f32, kind="ExternalOutput").ap()

    k1T = din("k1T", [C, N])
    v1T = din("v1T", [C, N])
    k2T = din("k2T", [C, N])
    v2T = din("v2T", [C, N])
    wk1 = din("wk1", [C, 128])   # column-duplicated [wk|wk]
    wk2 = din("wk2", [C, 128])
    wv1 = din("wv1", [C, D])
    wv2 = din("wv2", [C, D])
    bk1 = din("bk1", [128, 1], f32)  # row-duplicated
    bk2 = din("bk2", [128, 1], f32)
    wo1 = din("wo1", [128, C])  # row-duplicated [wo; wo]
    wo2 = din("wo2", [128, C])

    o1pT = dout("o1pT", [C, N])
    o2pT = dout("o2pT", [C, N])
    den1 = dout("den1", [128, MT])   # den1[m] at [m%128, m//128]
    den2 = dout("den2", [128, MT])

    with tile.TileContext(nc) as tc:
        with (
            tc.tile_pool(name="const", bufs=1) as constp,
            tc.tile_pool(name="raw", bufs=8) as rawp,
            tc.tile_pool(name="eblk", bufs=8) as ep,
            tc.tile_pool(name="outp", bufs=3) as outp,
            tc.tile_pool(name="spsum", bufs=2, space="PSUM") as spsum,
            tc.tile_pool(name="opsum", bufs=4, space="PSUM") as opsum,
        ):
            # ---- PE warm-up: ~7us of dummy matmuls so the HAM clock-gate
            # reaches K=8/8 (2.4 GHz) before the real work arrives ----
            warm = constp.tile([128, 512], bf16, tag="warm")
            nc.gpsimd.memset(warm[:], 0.0)
            wps = opsum.tile([128, 512], f32, tag="op", name="warm_ps")
            for _ in range(16):
                nc.tensor.matmul(wps[:], warm[:, 0:128], warm[:], start=True, stop=True)

            # ---- load weights ----
            w_sb = {}
            for name, drt, w in (("wk1", wk1, 128), ("wk2", wk2, 128),
                                 ("wv1", wv1, D), ("wv2", wv2, D)):
                t = constp.tile([128, 2, w], bf16, tag=name)
                for ct in range(2):
                    nc.sync.dma_start(out=t[:, ct, :], in_=drt[ct * 128:(ct + 1) * 128, :])
                w_sb[name] = t
            bk1_sb = constp.tile([128, 1], f32, tag="bk1")
            nc.sync.dma_start(out=bk1_sb[:], in_=bk1[:])
            bk2_sb = constp.tile([128, 1], f32, tag="bk2")
            nc.sync.dma_start(out=bk2_sb[:], in_=bk2[:])
            wo1_sb = constp.tile([128, C], bf16, tag="wo1")
            nc.sync.dma_start(out=wo1_sb[:], in_=wo1[:])
            wo2_sb = constp.tile([128, C], bf16, tag="wo2")
            nc.sync.dma_start(out=wo2_sb[:], in_=wo2[:])

            # ---- k projections: [128, 4096] bf16, data duplicated on both
            # partition halves (weights are column-duplicated) ----
            def k_proj(rawT, w, b_sb, tagbase):
                # chunked tiles so score matmuls can start before the whole
                # projection finishes (Tile deps are per-tile)
                full = []
                for j in range(8):
                    raw = rawp.tile([128, 2, 512], bf16, tag="raw")
                    for ct in range(2):
                        nc.sync.dma_start(
                            out=raw[:, ct, :],
                            in_=rawT[ct * 128:(ct + 1) * 128, j * 512:(j + 1) * 512],
                        )
                    ps = opsum.tile([128, 512], f32, tag="op")
                    for ct in range(2):
                        nc.tensor.matmul(
                            ps[:], w[:, ct, :], raw[:, ct, :],
                            start=(ct == 0), stop=(ct == 1),
                        )
                    fullj = constp.tile([128, 512], bf16, tag=f"{tagbase}_full{j}")
                    nc.vector.tensor_scalar_add(fullj[:], ps[:], b_sb[:])
                    full.append(fullj)
                return full

            k1_full = k_proj(k1T, w_sb["wk1"], bk1_sb, "k1p")
            k2_full = k_proj(k2T, w_sb["wk2"], bk2_sb, "k2p")

            # ---- v projections: [128, 32, 64] bf16 ----
            def v_proj(rawT, w, tagbase):
                vaug = constp.tile([128, MT, D], bf16, tag=tagbase)
                for j in range(8):
                    raw = rawp.tile([128, 2, 512], bf16, tag="raw")
                    for ct in range(2):
                        nc.sync.dma_start(
                            out=raw[:, ct, :],
                            in_=rawT[ct * 128:(ct + 1) * 128, j * 512:(j + 1) * 512],
                        )
                    for k in range(4):
                        nt = j * 4 + k
                        ps = opsum.tile([128, D], f32, tag="op")
                        for ct in range(2):
                            nc.tensor.matmul(
                                ps[:], raw[:, ct, k * 128:(k + 1) * 128],
                                w[:, ct, :],
                                start=(ct == 0), stop=(ct == 1),
                            )
                        nc.vector.tensor_copy(vaug[:, nt, :D], ps[:])
                return vaug


            # ---- one softmax direction ----
            def attention_pass(kP, kF, vaug, oT_tag, proj=None):
                """E[p, f] = exp(SCALE * kP[:,p]^T kF[:,f]); oT = vaug^T E.

                Score matmuls run 2x row-packed (64x128 tiles T0/T8). The exp
                accum_out sums E along the free axis, which is the OTHER
                direction's softmax denominator (both passes share the fully
                biased S). PV matmuls run 2x column-packed: even partition
                tiles accumulate into PSUM rows 0:64, odd into 64:128; the
                halves are summed later inside the output projection.
                """
                oT = []
                acc = constp.tile([128, MT, NCHUNK], f32, tag=oT_tag + "_acc")
                pvq = []

                def emit_pv(job):
                    jj, po, eblk, sub = job
                    for c in range(2):
                        for mtl in range(SUB):
                            mt = sub * SUB + mtl
                            half = (mt % 2) * 64
                            nc.tensor.matmul(
                                po[c][mt % 2][half:half + 64, :], vaug[:, mt, :],
                                eblk[:, mtl, c * 512:(c + 1) * 512],
                                start=(mt < 2), stop=(mt >= MT - 2),
                            )
                    if sub == 1:
                        for c in range(2):
                            oc = constp.tile([128, 512], bf16,
                                             tag=f"{oT_tag}_o{2*jj+c}")
                            nc.vector.tensor_copy(oc[0:64, :], po[c][0][0:64, :])
                            nc.vector.tensor_copy(oc[64:128, :], po[c][1][64:128, :])
                            oT.append(oc)

                for j in range(NCHUNK):
                    po = [[opsum.tile([128, 512], f32, tag="op",
                                      name=f"po_{oT_tag}_{j}_{c}_{h}")
                           for h in range(2)] for c in range(2)]
                    for sub in range(MT // SUB):
                        eblk = ep.tile([128, SUB, NBLK], bf16, tag="eblk")
                        for pair in range(SUB // 2):
                            mtA = sub * SUB + 2 * pair
                            mtB = mtA + 1
                            psA = spsum.tile([128, NBLK], f32, tag="sp")
                            psB = spsum.tile([128, NBLK], f32, tag="sp")
                            for ps, mt, lo, hi in ((psA, mtA, 0, 64), (psB, mtB, 64, 128)):
                                for c in range(2):
                                    nc.tensor.matmul(
                                        ps[:, c * 512:(c + 1) * 512],
                                        kP[mt // 4][lo:hi, (mt % 4) * 128:(mt % 4 + 1) * 128],
                                        kF[2 * j + c][lo:hi, :],
                                        start=True, stop=True,
                                    )
                            nc.scalar.activation(eblk[:, 2 * pair, :], psA[:], Exp,
                                                 scale=SCALE,
                                                 accum_out=acc[:, mtA, j:j + 1])
                            nc.scalar.activation(eblk[:, 2 * pair + 1, :], psB[:], Exp,
                                                 scale=SCALE,
                                                 accum_out=acc[:, mtB, j:j + 1])
                        pvq.append((j, po, eblk, sub))
                        if len(pvq) > 1:
                            emit_pv(pvq.pop(0))
                while pvq:
                    emit_pv(pvq.pop(0))
                # reduce the per-block accum partials -> other direction's den
                accr = constp.tile([128, MT], f32, tag=oT_tag + "_accr")
                nc.vector.tensor_reduce(accr[:], acc[:], mybir.AxisListType.X,
                                        mybir.AluOpType.add)
                return oT, accr

            # ---- output projections (per-head slice of wo); the two PV
            # halves (PSUM rows 0:64 / 64:128) are summed here via two
            # accumulating row-group matmuls against row-duplicated wo ----
            def out_proj(oT, accr, wo_sb, outdr, dendr):
                for j in range(8):
                    for ct in range(2):
                        pp = opsum.tile([128, 512], f32, tag="op")
                        nc.tensor.matmul(
                            pp[:], wo_sb[0:64, ct * 128:(ct + 1) * 128],
                            oT[j][0:64, :], start=True, stop=False,
                        )
                        nc.tensor.matmul(
                            pp[:], wo_sb[64:128, ct * 128:(ct + 1) * 128],
                            oT[j][64:128, :], start=False, stop=True,
                        )
                        ot = outp.tile([128, 512], f32, tag="out")
                        nc.vector.tensor_copy(ot[:], pp[:])
                        nc.sync.dma_start(
                            out=outdr[ct * 128:(ct + 1) * 128, j * 512:(j + 1) * 512],
                            in_=ot[:],
                        )
                nc.sync.dma_start(out=dendr[:], in_=accr[:])

            v1_aug = v_proj(v1T, w_sb["wv1"], "v1aug")
            v2_aug = v_proj(v2T, w_sb["wv2"], "v2aug")

            # o2 pass: E[m, n]; its exp accums give den1 (sum over n)
            o2T, acc1 = attention_pass(k2_full, k1_full, v2_aug, "o2T")
            out_proj(o2T, acc1, wo2_sb, o2pT, den1)
            # o1 pass: E[n, m]; its exp accums give den2 (sum over m)
            o1T, acc2 = attention_pass(k1_full, k2_full, v1_aug, "o1T")
            out_proj(o1T, acc2, wo1_sb, o1pT, den2)

    nc.compile()
    return nc


def _get_nc():
    if "nc" not in _cache:
        _cache["nc"] = _build_module()
    return _cache["nc"]


def kernel(k1, v1, k2, v2,
           wk1_w, wk1_b, wv1_w, wv1_b,
           wk2_w, wk2_b, wv2_w, wv2_b,
           wo1_w, wo1_b, wo2_w, wo2_b):
    import ml_dtypes
    from concourse.bass_utils import run_bass_kernel_spmd

    nc = _get_nc()

    f = np.float32
    bf = ml_dtypes.bfloat16
    k1T = np.ascontiguousarray(np.asarray(k1, f).T).astype(bf)
    v1T = np.ascontiguousarray(np.asarray(v1, f).T).astype(bf)
    k2T = np.ascontiguousarray(np.asarray(k2, f).T).astype(bf)
    v2T = np.ascontiguousarray(np.asarray(v2, f).T).astype(bf)

    def dup2(a):  # [C, D] -> [C, 128] column-duplicated
        return np.ascontiguousarray(np.concatenate([a, a], axis=1))

    in_maps = []
    for h in range(NCORES):
        sl = slice(h * D, (h + 1) * D)
        in_maps.append({
            "k1T": k1T, "v1T": v1T, "k2T": k2T, "v2T": v2T,
            "wk1": dup2(np.asarray(wk1_w, f)[:, sl]).astype(bf),
            "wv1": np.ascontiguousarray(np.asarray(wv1_w, f)[:, sl]).astype(bf),
            "wk2": dup2(np.asarray(wk2_w, f)[:, sl]).astype(bf),
            "wv2": np.ascontiguousarray(np.asarray(wv2_w, f)[:, sl]).astype(bf),
            "bk1": np.ascontiguousarray(np.tile(np.asarray(wk1_b, f)[sl].reshape(D, 1), (2, 1))),
            "bk2": np.ascontiguousarray(np.tile(np.asarray(wk2_b, f)[sl].reshape(D, 1), (2, 1))),
            "wo1": np.ascontiguousarray(np.concatenate([np.asarray(wo1_w, f)[sl, :]] * 2, axis=0)).astype(bf),
            "wo2": np.ascontiguousarray(np.concatenate([np.asarray(wo2_w, f)[sl, :]] * 2, axis=0)).astype(bf),
        })

    res = run_bass_kernel_spmd(nc, in_maps, list(range(NCORES)))
    _cache["last_result"] = res

    out1 = np.zeros((N, C), np.float32)
    out2 = np.zeros((N, C), np.float32)
    for h in range(NCORES):
        rh = res.results[h]
        den1v = rh["den1"].T.reshape(N)   # den1[m], m = mt*128 + p
        den2v = rh["den2"].T.reshape(N)   # den2[n]
        out1 += (rh["o1pT"] / den1v[None, :]).T
        out2 += (rh["o2pT"] / den2v[None, :]).T
    # v-bias and output bias constants (v-bias commutes through softmax).
    out1 += np.asarray(wv1_b, f) @ np.asarray(wo1_w, f) + np.asarray(wo1_b, f)
    out2 += np.asarray(wv2_b, f) @ np.asarray(wo2_w, f) + np.asarray(wo2_b, f)
    return out1, out2
